# revision 31
# baseline (speedup 1.0000x reference)
"""BiLSTM encoder Trainium2 kernel (8-core SPMD, batch-data-parallel).

Strategy (v2)
-------------
- Shard batch B=128 across 8 cores (16 per core); replicate weights.
- Host folds the input projection into the gate projection
  (pre = x @ (Wih @ W_proj)^T + bias) and pre-transposes x to the
  [D, rows] layout the GEMM needs (both forward and time-reversed
  copies), so the device does no casts/transposes in the GEMM phase.
- Device per core:
    GEMM: pre[t-block] = Wc^T-tiles @ xt-chunk, bias added with a K=1
      ones-matmul in PSUM (PE only); PSUM->SBUF bf16 copies on Act
      (AF.Copy shares the tanh table, so no table reloads) spread one
      group per 2 steps so no sequencer stalls behind them.
      pre is stored gate-major with the two directions interleaved:
      cols of block t = [i_f i_b | f_f f_b | o_f o_b | g_f g_b] x 16.
      The backward direction uses the time-reversed xt so its pre lands
      at scan position s = L-1-t with positive strides.
    Scan (combined f+b per step): ONE identity matmul loads pre block
      into PSUM, 8 small Whh matmuls accumulate both directions' gate
      recurrent terms, ONE [128,128] tanh (i/f/o rows pre-scaled by 0.5
      so sigmoid(x) = 0.5*tanh(x/2)+0.5 fuses into the DVE ops), ONE
      merged affine_mul_reduce computes u=sig(i)*tanh(g) and
      v=sig(f)*c together (c lives adjacent to the g-gates in a
      persistent work tile), one add, ONE [128,32] tanh(c), 2 amr ops
      write h_f/h_b (h_f first so PE starts earlier).
    LayerNorm: interleaved with the scan as 128-row chunks become
      ready; xbar-DMA transpose back to row space (issued from the SP
      sequencer), bn_stats/bn_aggr + rstd=(var+eps)^-0.5 on DVE (bit
      trick + one Newton step; no Act-table switch), normalize on
      Pool, output DMA on SP; the finish is split into 4 single-step
      parts so the small DVE ops never clog the in-order DVE pipe.
"""

import sys

for _p in ("/opt/trn_rl_repo", "/opt/pypackages"):
    if _p not in sys.path:
        sys.path.insert(0, _p)

from contextlib import ExitStack

import ml_dtypes
import numpy as np

import concourse.bacc as bacc
import concourse.mybir as mybir
import concourse.tile as tile
from concourse.bass_utils import run_bass_kernel_spmd

BF = mybir.dt.bfloat16
F32 = mybir.dt.float32
AF = mybir.ActivationFunctionType
ALU = mybir.AluOpType

H = 128
DD = 256
G4 = 512
N_CORES = 8
EPS = 1e-5

_BUILD_CACHE = {}


def build_nc(L=512, BL=16, n_cores=N_CORES):
    key = (L, BL, n_cores)
    if key in _BUILD_CACHE:
        return _BUILD_CACHE[key]
    R = BL * L               # rows per core per direction
    CH = 512                 # GEMM chunk columns (x rows)
    NCH = R // CH            # chunks
    TCH = CH // BL           # scan steps covered per chunk
    NLN = R // 128           # layernorm chunks
    LNT = 128 // BL          # time steps per LN chunk
    LA = 4                   # identity-matmul prefetch lookahead
    nc = bacc.Bacc("TRN2", target_bir_lowering=False, debug=False, num_devices=n_cores)

    xt_d = nc.dram_tensor("xt", [2, H, R], BF, kind="ExternalInput").ap()
    xtr_d = nc.dram_tensor("xtr", [2, H, R], BF, kind="ExternalInput").ap()
    wc_d = {
        d: [
            nc.dram_tensor(f"wc_{d}{k}", [128, G4], BF, kind="ExternalInput").ap()
            for k in range(2)
        ]
        for d in "fb"
    }
    whh_d = {
        d: nc.dram_tensor(f"whh_{d}", [128, G4], BF, kind="ExternalInput").ap()
        for d in "fb"
    }
    bc_d = {
        d: nc.dram_tensor(f"bc_{d}", [4, 128], BF, kind="ExternalInput").ap()
        for d in "fb"
    }
    ident_d = nc.dram_tensor("ident", [128, 128], BF, kind="ExternalInput").ap()
    y = nc.dram_tensor("y", [BL, L, 2 * H], F32, kind="ExternalOutput").ap()
    y3 = y.rearrange("b l f -> l b f")

    with tile.TileContext(nc) as tc:
        with ExitStack() as ctx:
            wpool = ctx.enter_context(tc.tile_pool(name="w", bufs=1))
            prepool = ctx.enter_context(tc.tile_pool(name="pre", bufs=1))
            hspool = ctx.enter_context(tc.tile_pool(name="hs", bufs=1))
            stpool = ctx.enter_context(tc.tile_pool(name="state", bufs=1))

            # chunk-0 x tiles first so their DMAs lead the SP queue and
            # the first GEMM chunk's data is in flight during weight loads
            xtp = ctx.enter_context(tc.tile_pool(name="xt", bufs=6))
            gemm_x = {}

            def emit_gemm_dma(ch):
                xts = {}
                for d, src in (("f", xt_d), ("b", xtr_d)):
                    for kt in range(2):
                        t_ = xtp.tile([128, CH], BF, name=f"x{d}{kt}")
                        nc.sync.dma_start(t_[:], src[kt][:, ch * CH : (ch + 1) * CH])
                        xts[d, kt] = t_
                gemm_x[ch] = xts

            emit_gemm_dma(0)

            wc_sb = {}
            whh_sb = {}
            bc_sb = {}
            for d in "fb":
                wc_sb[d] = []
                for k in range(2):
                    t_ = wpool.tile([128, G4], BF, tag=f"wc{d}{k}")
                    nc.sync.dma_start(t_[:], wc_d[d][k])
                    wc_sb[d].append(t_)
                t_ = wpool.tile([128, G4], BF, tag=f"whh{d}")
                nc.sync.dma_start(t_[:], whh_d[d])
                whh_sb[d] = t_
                bc_sb[d] = []
                for m in range(4):
                    t_ = wpool.tile([1, 128], BF, tag=f"bc{d}{m}")
                    nc.sync.dma_start(t_[:], bc_d[d][m : m + 1])
                    bc_sb[d].append(t_)
            ident = wpool.tile([128, 128], BF, tag="ident")
            nc.sync.dma_start(ident[:], ident_d)
            ones = wpool.tile([1, CH], BF, tag="ones")
            nc.gpsimd.memset(ones[:], 1.0)

            # pre: [128, L*128] bf16, block t cols = (gate, dir, batch)
            pre = prepool.tile([128, L * 128], BF, tag="pre", name="pre")
            pre_ap = pre[:]
            pre_v = pre_ap.rearrange("p (t g e b) -> p t g e b", g=4, e=2, b=BL)
            # hs: forward dir at [0, R), backward at [R, 2R)
            hs = hspool.tile([128, 2 * R], BF, tag="hs", name="hs")
            hs_ap = hs[:]
            # persistent scan work tile: ta gates at [0:128], c at
            # [128:160], th at [160:192] — c adjacent to the g-gate block
            # so one affine_mul_reduce computes u and v together
            wk = stpool.tile([128, 192], F32, tag="wk", name="wk")
            nc.vector.memset(wk[:, 8 * BL : 10 * BL], 0.0)

            gpsum = ctx.enter_context(tc.tile_pool(name="gpsum", bufs=2, space="PSUM"))
            spsum = ctx.enter_context(tc.tile_pool(name="spsum", bufs=6, space="PSUM"))
            uvp = ctx.enter_context(tc.tile_pool(name="uv", bufs=2))
            junkp = ctx.enter_context(tc.tile_pool(name="junk", bufs=8))
            xrp = ctx.enter_context(tc.tile_pool(name="xr", bufs=6))
            bnp = ctx.enter_context(tc.tile_pool(name="bn", bufs=4))
            statp = ctx.enter_context(tc.tile_pool(name="stat", bufs=6))
            otp = ctx.enter_context(tc.tile_pool(name="ot", bufs=4))

            # ---------------- GEMM ----------------
            gemm_pg = {}

            def emit_gemm_mms(ch, gi):
                xts = gemm_x[ch]
                di, m = divmod(gi, 4)
                d = "fb"[di]
                pg = gpsum.tile([128, CH], F32, name="pg")
                nc.tensor.matmul(
                    pg[:],
                    wc_sb[d][0][:, m * 128 : (m + 1) * 128],
                    xts[d, 0][:],
                    start=True,
                    stop=False,
                )
                nc.tensor.matmul(
                    pg[:],
                    wc_sb[d][1][:, m * 128 : (m + 1) * 128],
                    xts[d, 1][:],
                    start=False,
                    stop=False,
                )
                nc.tensor.matmul(
                    pg[:],
                    bc_sb[d][m][:],
                    ones[0:1, :],
                    start=False,
                    stop=True,
                )
                gemm_pg[ch, gi] = pg

            def emit_gemm_copy(ch, gi, eng):
                # GPSIMD/Pool cannot read PSUM; Act's Copy shares the tanh
                # table (no reload) and one ~600ns copy per 2 steps hides
                # in the gaps between the scan's two tanh ops.
                di, m = divmod(gi, 4)
                pg = gemm_pg.pop((ch, gi))
                dst = pre_v[:, ch * TCH : (ch + 1) * TCH, m, di, :]
                src_v = pg[:].rearrange("p (t b) -> p t b", b=BL)
                if eng == "dve":
                    nc.vector.tensor_copy(dst, src_v)
                else:
                    nc.scalar.activation(dst, src_v, AF.Copy)

            def emit_gemm_chunk(ch, first=False):
                if ch not in gemm_x:
                    emit_gemm_dma(ch)
                for gi in range(8):
                    emit_gemm_mms(ch, gi)
                    # prologue chunk alternates Act/DVE to shorten the head
                    eng = ("act" if gi % 2 == 0 else "dve") if first else "act"
                    emit_gemm_copy(ch, gi, eng)
                del gemm_x[ch]

            # chunk k>=1: DMAs at step 32k-26, one matmul-group+copy every
            # SECOND step over [32k-18, 32k-4] (spread so the sequencers
            # never stall behind DMA-waiting GEMM work)
            GEMM_DMA_LEAD = 26
            GEMM_MM_LEAD = 18

            def emit_gemm_piece(s):
                kd = (s + GEMM_DMA_LEAD) // TCH
                if 1 <= kd < NCH and (s + GEMM_DMA_LEAD) % TCH == 0:
                    emit_gemm_dma(kd)
                km = (s + GEMM_MM_LEAD) // TCH
                if 1 <= km < NCH and km in gemm_x:
                    r = (s + GEMM_MM_LEAD) % TCH
                    if r < 16 and r % 2 == 0:
                        gi = r // 2
                        emit_gemm_mms(km, gi)
                        emit_gemm_copy(km, gi, "act")
                        if gi == 7:
                            del gemm_x[km]

            # ---------------- scan ----------------
            ps_tiles = {}

            def emit_ident(s):
                ps = spsum.tile([128, 8 * BL], F32, name="ps")
                nc.tensor.matmul(
                    ps[:],
                    ident[:],
                    pre_ap[:, s * 128 : (s + 1) * 128],
                    start=True,
                    stop=True,
                )
                ps_tiles[s] = ps

            # -------------- layernorm --------------
            def emit_ln_xbar(cc):
                xr = xrp.tile([128, 2 * H], BF, name="xr")
                nc.sync.dma_start_transpose(
                    xr[:, 0:H], hs_ap[:, cc * 128 : (cc + 1) * 128]
                )
                nc.sync.dma_start_transpose(
                    xr[:, H : 2 * H], hs_ap[:, R + cc * 128 : R + (cc + 1) * 128]
                )
                return xr

            I32 = mybir.dt.uint32

            # The LN finish is split into 4 parts emitted on separate scan
            # steps so its small DVE ops never bunch up and stall the
            # in-order DVE pipe. rstd = (var+eps)^-1/2 entirely on DVE:
            # reciprocal-approx of vh=(var+eps)/2, sqrt-magic bit seed
            # (constant shifted to absorb the 1/sqrt(2)), one Newton
            # iteration -> ~3e-3 rel err. (The Act engine's Sqrt lives in
            # a different activation table: a 1.3us reload mid-scan.)
            def ln_parts(cc, xr):
                st = {}

                def pA():
                    st["bn6"] = bnp.tile([128, 6], F32, name="bn6")
                    nc.vector.bn_stats(st["bn6"][:], xr[:])
                    st["ag2"] = bnp.tile([128, 2], F32, name="ag2")
                    nc.vector.bn_aggr(st["ag2"][:], st["bn6"][:])

                def pB():
                    st["vh"] = statp.tile([128, 1], F32, tag="vh", name="vh")
                    nc.vector.tensor_scalar(
                        st["vh"][:], st["ag2"][:, 1:2], EPS, 0.5, ALU.add, ALU.mult
                    )
                    rv = statp.tile([128, 1], F32, tag="rv", name="rv")
                    nc.vector.reciprocal_approx_fast(rv[:], st["vh"][:])
                    sh = statp.tile([128, 1], F32, tag="sh", name="sh")
                    nc.vector.tensor_scalar(
                        sh[:].bitcast(I32), rv[:].bitcast(I32), 1, None,
                        ALU.logical_shift_right,
                    )
                    # sign bit set in the magic constant: the Newton step
                    # below computes (q-1.5)*x0 (HW stt operand order is
                    # (in0 op0 scalar)), so a NEGATIVE seed converges to
                    # +rstd in one iteration
                    st["x0"] = statp.tile([128, 1], F32, tag="x0", name="x0")
                    nc.vector.tensor_scalar(
                        st["x0"][:].bitcast(I32), sh[:].bitcast(I32), 0x9F7D1DF5,
                        None, ALU.add,
                    )

                def pC():
                    q = statp.tile([128, 1], F32, tag="q", name="q")
                    nc.vector.tensor_mul(q[:], st["x0"][:], st["x0"][:])
                    nc.vector.tensor_mul(q[:], q[:], st["vh"][:])
                    nc.vector.scalar_tensor_tensor(
                        st["x0"][:], q[:], 1.5, st["x0"][:], ALU.subtract, ALU.mult
                    )
                    st["nmr"] = statp.tile([128, 1], F32, tag="nmr", name="nmr")
                    nc.vector.scalar_tensor_tensor(
                        st["nmr"][:], st["ag2"][:, 0:1], -1.0, st["x0"][:], ALU.mult,
                        ALU.mult,
                    )

                def pD():
                    nmr = st["nmr"]
                    ot = otp.tile([128, 2 * H], F32, name="ot")
                    nc.gpsimd.tensor_scalar(
                        ot[:], xr[:], st["x0"][:], nmr[:], ALU.mult, ALU.add
                    )
                    nc.sync.dma_start(y3[cc * LNT : (cc + 1) * LNT], ot[:])

                return [pA, pB, pC, pD]

            def s_ready(cc):
                return max(LNT * cc + LNT - 1, L - 1 - LNT * cc)

            ln_at = {}
            for cc in range(NLN):
                ln_at.setdefault(s_ready(cc), []).append(cc)
            LN_LAG = 4  # steps between xbar issue and stats/normalize

            def emit_step(s):
                emit_gemm_piece(s)
                if s + LA < L:
                    emit_ident(s + LA)
                ps = ps_tiles.pop(s)
                if s > 0:
                    h_f = hs_ap[:, (s - 1) * BL : s * BL]
                    h_b = hs_ap[:, R + (L - s) * BL : R + (L - s + 1) * BL]
                    for g in range(4):
                        nc.tensor.matmul(
                            ps[:, g * 2 * BL : g * 2 * BL + BL],
                            whh_sb["f"][:, g * 128 : (g + 1) * 128],
                            h_f,
                            start=False,
                            stop=True,
                            skip_group_check=True,
                        )
                    for g in range(4):
                        nc.tensor.matmul(
                            ps[:, g * 2 * BL + BL : (g + 1) * 2 * BL],
                            whh_sb["b"][:, g * 128 : (g + 1) * 128],
                            h_b,
                            start=False,
                            stop=True,
                            skip_group_check=True,
                        )
                # ta gates into wk[0:128]; one amr computes [u | v] since
                # in1 = [g-gates | c] is contiguous in wk
                nc.scalar.activation(wk[:, 0 : 8 * BL], ps[:], AF.Tanh)
                uv = uvp.tile([128, 4 * BL], F32, tag="uv", name="uv")
                ju = junkp.tile([128, 1], F32, name="ju")
                nc.vector.affine_mul_reduce(
                    uv[:], ju[:], wk[:, 0 : 4 * BL], wk[:, 6 * BL : 10 * BL], 0.5, 0.5
                )
                nc.vector.tensor_add(
                    wk[:, 8 * BL : 10 * BL], uv[:, 0 : 2 * BL], uv[:, 2 * BL : 4 * BL]
                )
                nc.scalar.activation(
                    wk[:, 10 * BL : 12 * BL], wk[:, 8 * BL : 10 * BL], AF.Tanh
                )
                jf = junkp.tile([128, 1], F32, name="jf")
                nc.vector.affine_mul_reduce(
                    hs_ap[:, s * BL : (s + 1) * BL],
                    jf[:],
                    wk[:, 4 * BL : 5 * BL],
                    wk[:, 10 * BL : 11 * BL],
                    0.5,
                    0.5,
                )
                jb = junkp.tile([128, 1], F32, name="jb")
                nc.vector.affine_mul_reduce(
                    hs_ap[:, R + (L - 1 - s) * BL : R + (L - s) * BL],
                    jb[:],
                    wk[:, 5 * BL : 6 * BL],
                    wk[:, 11 * BL : 12 * BL],
                    0.5,
                    0.5,
                )

            # prologue: first GEMM chunk + identity prefetch
            emit_gemm_chunk(0, first=True)
            for s in range(LA):
                emit_ident(s)

            ln_q = []  # (activation_step, [remaining parts])
            for s in range(L):
                emit_step(s)
                for cc in ln_at.get(s, []):
                    ln_q.append((s + LN_LAG, ln_parts(cc, emit_ln_xbar(cc))))
                # at most ONE part per step keeps the DVE pipe clear
                for ent in ln_q:
                    if ent[0] <= s and ent[1]:
                        ent[1].pop(0)()
                        break
                ln_q = [e for e in ln_q if e[1]]
            for _, parts in ln_q:
                for p in parts:
                    p()

    nc.compile()
    _BUILD_CACHE[key] = nc
    return nc


def _prep_weights(W_proj, b_proj, Wih, Whh, b):
    """Host-side: fold projection, permute gates to (i,f,o,g), pre-scale
    i/f/o rows by 0.5 (sigmoid-via-tanh trick), build lhsT layouts."""
    perm = np.r_[0:256, 384:512, 256:384]
    scale = np.concatenate([np.full(384, 0.5), np.ones(128)]).astype(np.float64)
    Wc = (Wih.astype(np.float64) @ W_proj.astype(np.float64))[perm] * scale[:, None]
    bc = (Wih.astype(np.float64) @ b_proj.astype(np.float64) + b.astype(np.float64))[
        perm
    ] * scale
    Whh_p = Whh[perm].astype(np.float64) * scale[:, None]
    bf16 = ml_dtypes.bfloat16
    WcT = np.ascontiguousarray(Wc.T.astype(np.float32).astype(bf16))  # [D, 4H]
    WhhT = np.ascontiguousarray(Whh_p.T.astype(np.float32).astype(bf16))  # [H, 4H]
    bc4 = np.ascontiguousarray(bc.astype(np.float32).reshape(4, 128).astype(bf16))
    return WcT, WhhT, bc4


def _prep_x(shard):
    """[BL, L, D] f32 -> (xt, xtr) each [2, 128, L*BL] bf16; xtr is
    time-reversed so the backward GEMM lands at scan positions."""
    bf16 = ml_dtypes.bfloat16
    BLc, L, D = shard.shape
    xs = shard.transpose(2, 1, 0)  # [D, L, BL]
    xt = xs.reshape(D, L * BLc).astype(bf16)
    xtr = xs[:, ::-1, :].reshape(D, L * BLc).astype(bf16)
    return (
        np.ascontiguousarray(xt.reshape(2, 128, L * BLc)),
        np.ascontiguousarray(xtr.reshape(2, 128, L * BLc)),
    )


def kernel(x, W_proj, b_proj, Wih_f, Whh_f, b_f, Wih_b, Whh_b, b_b, gamma, beta):
    x = np.asarray(x, dtype=np.float32)
    B, L, D = x.shape
    BL = B // N_CORES
    nc = build_nc(L=L, BL=BL)

    bf16 = ml_dtypes.bfloat16
    in_common = {"ident": np.eye(128, dtype=np.float32).astype(bf16)}
    for d, Wih, Whh, b in (("f", Wih_f, Whh_f, b_f), ("b", Wih_b, Whh_b, b_b)):
        WcT, WhhT, bc4 = _prep_weights(
            np.asarray(W_proj), np.asarray(b_proj), np.asarray(Wih), np.asarray(Whh),
            np.asarray(b),
        )
        in_common[f"wc_{d}0"] = np.ascontiguousarray(WcT[0:128])
        in_common[f"wc_{d}1"] = np.ascontiguousarray(WcT[128:256])
        in_common[f"whh_{d}"] = WhhT
        in_common[f"bc_{d}"] = bc4

    in_maps = []
    for i in range(N_CORES):
        xt, xtr = _prep_x(x[i * BL : (i + 1) * BL])
        in_maps.append({**in_common, "xt": xt, "xtr": xtr})
    res = run_bass_kernel_spmd(nc, in_maps, list(range(N_CORES)))
    out = np.concatenate([res.results[i]["y"] for i in range(N_CORES)], axis=0)

    gamma = np.asarray(gamma, dtype=np.float32)
    beta = np.asarray(beta, dtype=np.float32)
    if not (np.all(gamma == 1.0) and np.all(beta == 0.0)):
        out = out * gamma + beta
    return out.astype(np.float32)


if __name__ == "__main__":
    d = np.load("/root/problem/ref.npz")
    inp = {k: d[k] for k in d.files if k != "exp"}
    got = kernel(**inp)
    exp = d["exp"]
    rel = np.linalg.norm(got - exp) / np.linalg.norm(exp)
    print("rel fro:", rel, "maxabs:", np.abs(got - exp).max())
